# revision 12
# baseline (speedup 1.0000x reference)
"""Trainium2 kernel for CustomContextEncoderForQG (v3).

Host: the two BiLSTM layers (sequential recurrence, small batch) run on CPU.
Device: attention block (QKV projections + 10-head softmax attention +
residual) as a Bass/Tile SPMD kernel on 8 NeuronCores, data-parallel over
batch (2 sequences per core).

Layout strategy: everything stays in transposed [feature, seq] layout
end-to-end, so no PE transposes are needed:
  - Q.T, K.T computed as [d, seq] (d on partitions); epilogue (bias+scale)
    fused on ScalarE via activation(Identity, bias, scale)
  - V computed in natural [seq, d] layout directly (no bias; folded below)
  - scores computed as S.T [k, q] = K.T-slices.T @ Q.T (contract over d)
  - softmax over k = partition dim: exp fused with the per-partition key
    mask as the activation bias (no max subtraction -- scores are O(1)
    here and the -10000 mask underflows to exactly 0); denominator D via
    all-ones matmul (rows of the PSUM tile all equal D); 1/D via the
    single-op reciprocal_approx_fast
  - ctx.T [d, q] = V-slices.T @ E.T (contract over k), then
    out.T = ctx.T * (1/D) + (h.T + bv)  (bv folded analytically: softmax
    rows sum to 1, so P @ (V + 1 bv^T) = P @ V + bv; h.T + bv precomputed
    once per sequence on ScalarE)
  - head loop is software-pipelined: scores+exp of head i issue before the
    denominator/context matmuls of head i-1, so the PE never waits on exp

All matmuls run in fp16 (same PE rate as bf16, 8x finer mantissa -- keeps
max rel err vs the fp32 reference under 1e-2); accumulation is fp32 PSUM.
"""

import sys
import numpy as np

sys.path.insert(0, "/opt/trn_rl_repo")


B, S, D_MODEL, H, NHEADS = 16, 512, 768, 640, 10
D_ATT = 2 * H  # 1280
HEAD_DIM = D_ATT // NHEADS  # 128
N_CORES = 8
BPC = B // N_CORES  # 2 sequences per core
NK = D_ATT // 128  # 10 chunks of the 1280 dim
NSC = S // 128  # 4 seq chunks of 128
SCALE = float(1.0 / np.sqrt(HEAD_DIM))


def _sigmoid(x):
    return 1.0 / (1.0 + np.exp(-x))


def _lstm_dir(xp, Whh, lengths, reverse):
    # xp: [B,S,4H]; packed-sequence semantics (state frozen, output zeroed
    # for t >= length); torch gate order i,f,g,o.
    Bs, Ss, H4 = xp.shape
    Hh = H4 // 4
    WhhT = np.ascontiguousarray(Whh.T)
    h = np.zeros((Bs, Hh), np.float32)
    c = np.zeros((Bs, Hh), np.float32)
    out = np.zeros((Bs, Ss, Hh), np.float32)
    ts = range(Ss - 1, -1, -1) if reverse else range(Ss)
    for t in ts:
        g = xp[:, t] + h @ WhhT
        i = _sigmoid(g[:, :Hh])
        f = _sigmoid(g[:, Hh : 2 * Hh])
        gg = np.tanh(g[:, 2 * Hh : 3 * Hh])
        o = _sigmoid(g[:, 3 * Hh :])
        c2 = f * c + i * gg
        h2 = o * np.tanh(c2)
        valid = (t < lengths)[:, None]
        h = np.where(valid, h2, h)
        c = np.where(valid, c2, c)
        out[:, t] = np.where(valid, h, 0.0)
    return out


def _bilstm_layer(x, Wih, Whh, b, lengths):
    outs = []
    for d, rev in ((0, False), (1, True)):
        xp = x @ Wih[d].T + b[d]
        outs.append(_lstm_dir(xp, Whh[d], lengths, rev))
    return np.concatenate(outs, axis=-1)


def _attention_numpy(h, mask, Wq, bq, Wk, bk, Wv, bv):
    q = (h @ Wq.T + bq).reshape(B, S, NHEADS, HEAD_DIM)
    k = (h @ Wk.T + bk).reshape(B, S, NHEADS, HEAD_DIM)
    v = (h @ Wv.T + bv).reshape(B, S, NHEADS, HEAD_DIM)
    scores = np.einsum("bqhd,bkhd->bhqk", q, k) / np.float32(np.sqrt(HEAD_DIM))
    scores = scores + mask  # [B,1,1,S]
    scores = scores - scores.max(-1, keepdims=True)
    e = np.exp(scores)
    probs = e / e.sum(-1, keepdims=True)
    ctx = np.einsum("bhqk,bkhd->bqhd", probs, v).reshape(B, S, D_ATT)
    return h + ctx


_NC_CACHE = {}
_LAST_RES = None


def _build_attention_nc():
    import concourse.bacc as bacc
    import concourse.mybir as mybir
    from concourse import tile

    fp32 = mybir.dt.float32
    f16 = mybir.dt.float16
    AF = mybir.ActivationFunctionType
    OP = mybir.AluOpType

    # Bacc (not plain Bass) so nc.compile() below can legalize the module
    # (walrus allows at most one semaphore wait per engine instruction).
    nc = bacc.Bacc()
    # fp16 h.T for matmul inputs; fp32 h.T for the residual path
    hth_ext = nc.declare_dram_parameter("hth", [BPC, D_ATT, S], f16, isOutput=False)
    htf_ext = nc.declare_dram_parameter("htf", [BPC, D_ATT, S], fp32, isOutput=False)
    wqt_ext = nc.declare_dram_parameter("wqt", [D_ATT, D_ATT], f16, isOutput=False)
    wkt_ext = nc.declare_dram_parameter("wkt", [D_ATT, D_ATT], f16, isOutput=False)
    wvt_ext = nc.declare_dram_parameter("wvt", [D_ATT, D_ATT], f16, isOutput=False)
    # bqs = bq * SCALE (folded into the Q epilogue's activation bias)
    bqs_ext = nc.declare_dram_parameter("bqs", [128, NK], fp32, isOutput=False)
    bkt_ext = nc.declare_dram_parameter("bkt", [128, NK], fp32, isOutput=False)
    bvt_ext = nc.declare_dram_parameter("bvt", [128, NK], fp32, isOutput=False)
    maskt_ext = nc.declare_dram_parameter("maskt", [BPC, 128, NSC], fp32, isOutput=False)
    out_ext = nc.declare_dram_parameter("outt", [BPC, D_ATT, S], fp32, isOutput=True)

    V_OCHUNKS = [(0, 512), (512, 512), (1024, 256)]

    with tile.TileContext(nc) as tc:
        with (
            tc.tile_pool(name="wpool", bufs=1) as wpool,
            tc.tile_pool(name="const", bufs=1) as const,
            tc.tile_pool(name="maskp", bufs=2) as maskp,
            tc.tile_pool(name="hb", bufs=1) as hb,
            tc.tile_pool(name="hf", bufs=1) as hf,
            tc.tile_pool(name="h2", bufs=1) as h2p,
            tc.tile_pool(name="qk", bufs=1) as qk,
            tc.tile_pool(name="vp", bufs=1) as vp,
            tc.tile_pool(name="et", bufs=8) as etp,
            tc.tile_pool(name="em", bufs=2) as emp,
            tc.tile_pool(name="es", bufs=2) as esp,
            tc.tile_pool(name="rp", bufs=2) as rp,
            tc.tile_pool(name="op", bufs=3) as op,
            tc.tile_pool(name="ps", bufs=8, space="PSUM") as psp,
        ):
            ones = const.tile([128, 128], f16, tag="ones")
            nc.vector.memset(ones[:], 1.0)

            # --- DMA order is chosen so the earliest compute phase's inputs
            # (hth seq0 + wv for the V phase) arrive first ---
            hth = [[None] * NK for _ in range(BPC)]
            wsb = {"q": [], "k": [], "v": []}
            for kc in range(NK):
                t = hb.tile([128, S], f16, tag=f"hth0_{kc}")
                nc.sync.dma_start(out=t[:], in_=hth_ext[0, kc * 128 : (kc + 1) * 128, :])
                hth[0][kc] = t
                wt = wpool.tile([128, D_ATT], f16, tag=f"wv{kc}")
                nc.sync.dma_start(out=wt[:], in_=wvt_ext[kc * 128 : (kc + 1) * 128, :])
                wsb["v"].append(wt)
            for name, wext in (("q", wqt_ext), ("k", wkt_ext)):
                for kc in range(NK):
                    wt = wpool.tile([128, D_ATT], f16, tag=f"w{name}{kc}")
                    nc.sync.dma_start(out=wt[:], in_=wext[kc * 128 : (kc + 1) * 128, :])
                    wsb[name].append(wt)
            bqs = const.tile([128, NK], fp32, tag="bqs")
            nc.sync.dma_start(out=bqs[:], in_=bqs_ext[:, :])
            bkt = const.tile([128, NK], fp32, tag="bkt")
            nc.sync.dma_start(out=bkt[:], in_=bkt_ext[:, :])
            bvt = const.tile([128, NK], fp32, tag="bvt")
            nc.sync.dma_start(out=bvt[:], in_=bvt_ext[:, :])
            masks = []
            for b in range(BPC):
                m = maskp.tile([128, NSC], fp32, tag="mask")
                nc.sync.dma_start(out=m[:], in_=maskt_ext[b])
                masks.append(m)
            htf = [[None] * NK for _ in range(BPC)]
            for kc in range(NK):
                t = hf.tile([128, S], fp32, tag=f"htf0_{kc}")
                nc.sync.dma_start(out=t[:], in_=htf_ext[0, kc * 128 : (kc + 1) * 128, :])
                htf[0][kc] = t
            for kc in range(NK):
                t = hb.tile([128, S], f16, tag=f"hth1_{kc}")
                nc.sync.dma_start(
                    out=t[:], in_=hth_ext[1, kc * 128 : (kc + 1) * 128, :]
                )
                hth[1][kc] = t
            for kc in range(NK):
                t = hf.tile([128, S], fp32, tag=f"htf1_{kc}")
                nc.sync.dma_start(out=t[:], in_=htf_ext[1, kc * 128 : (kc + 1) * 128, :])
                htf[1][kc] = t

            for b in range(BPC):
                # --- V in natural [seq, d] layout (no bias; folded into hb2) ---
                v_tiles = []
                for sc in range(NSC):
                    vt = vp.tile([128, D_ATT], f16, tag=f"v{sc}")
                    for o0, on in V_OCHUNKS:
                        ps = psp.tile([128, S], fp32, tag="ps")
                        for kc in range(NK):
                            nc.tensor.matmul(
                                ps[:, :on],
                                hth[b][kc][:, sc * 128 : (sc + 1) * 128],
                                wsb["v"][kc][:, o0 : o0 + on],
                                start=(kc == 0),
                                stop=(kc == NK - 1),
                            )
                        nc.vector.tensor_copy(out=vt[:, o0 : o0 + on], in_=ps[:, :on])
                    v_tiles.append(vt)

                def proj_group(which, wlist, btile, scale, mc, tag):
                    """one [d-chunk, seq] projection column-block + ACT epilogue"""
                    ps = psp.tile([128, S], fp32, tag="ps")
                    for kc in range(NK):
                        nc.tensor.matmul(
                            ps[:],
                            wlist[kc][:, mc * 128 : (mc + 1) * 128],
                            hth[b][kc][:],
                            start=(kc == 0),
                            stop=(kc == NK - 1),
                        )
                    ob = qk.tile([128, S], f16, tag=tag)
                    nc.scalar.activation(
                        out=ob[:], in_=ps[:], func=AF.Identity,
                        bias=btile[:, mc : mc + 1], scale=scale,
                    )
                    return ob

                def head_front(hd, qt_h, kt_h):
                    """scores + exp + E-presum for head hd"""
                    ets = []
                    for kc4 in range(NSC):
                        sps = psp.tile([128, S], fp32, tag="ps")
                        nc.tensor.matmul(
                            sps[:],
                            kt_h[:, kc4 * 128 : (kc4 + 1) * 128],
                            qt_h[:],
                            start=True,
                            stop=True,
                        )
                        et = etp.tile([128, S], f16, tag="et")
                        nc.scalar.activation(
                            out=et[:], in_=sps[:], func=AF.Exp,
                            bias=masks[b][:, kc4 : kc4 + 1], scale=1.0,
                        )
                        ets.append(et)
                    # E-presum on DVE so the denominator needs one matmul
                    t1 = emp.tile([128, S], f16, tag="etm")
                    nc.vector.tensor_tensor(
                        out=t1[:], in0=ets[0][:], in1=ets[1][:], op=OP.add
                    )
                    t2 = emp.tile([128, S], f16, tag="etm")
                    nc.vector.tensor_tensor(
                        out=t2[:], in0=ets[2][:], in1=ets[3][:], op=OP.add
                    )
                    esum = esp.tile([128, S], f16, tag="esum")
                    nc.vector.tensor_tensor(
                        out=esum[:], in0=t1[:], in1=t2[:], op=OP.add
                    )
                    return ets, esum

                def head_tail(hd, ets, esum, hb2_t):
                    """context, denominator, epilogue + store for head hd"""
                    cps = psp.tile([128, S], fp32, tag="ps")
                    for kc4 in range(NSC):
                        nc.tensor.matmul(
                            cps[:],
                            v_tiles[kc4][:, hd * 128 : (hd + 1) * 128],
                            ets[kc4][:],
                            start=(kc4 == 0), stop=(kc4 == NSC - 1),
                        )
                    dps = psp.tile([128, S], fp32, tag="ps")
                    nc.tensor.matmul(dps[:], ones[:], esum[:], start=True, stop=True)
                    r = rp.tile([128, S], fp32, tag="r")
                    nc.vector.reciprocal_approx_fast(out=r[:], in_=dps[:])
                    ot = op.tile([128, S], fp32, tag="ot")
                    nc.vector.tensor_tensor(
                        out=ot[:], in0=cps[:], in1=r[:], op=OP.mult
                    )
                    nc.vector.tensor_tensor(
                        out=ot[:], in0=ot[:], in1=hb2_t[:], op=OP.add
                    )
                    nc.sync.dma_start(
                        out=out_ext[b, hd * 128 : (hd + 1) * 128, :], in_=ot[:]
                    )

                # --- per-head: project this head's Q/K column-blocks, then
                # scores+exp; the previous head's denominator/context matmuls
                # interleave so the PE never waits on the Scalar engine ---
                prev = None
                for hd in range(NHEADS):
                    qt_h = proj_group("qT", wsb["q"], bqs, SCALE, hd, f"qT{hd}")
                    kt_h = proj_group("kT", wsb["k"], bkt, 1.0, hd, f"kT{hd}")
                    # h.T + bv for this head's residual (ACT, fp32)
                    h2t = h2p.tile([128, S], fp32, tag=f"hb2_{hd}")
                    nc.scalar.activation(
                        out=h2t[:], in_=htf[b][hd][:], func=AF.Identity,
                        bias=bvt[:, hd : hd + 1], scale=1.0,
                    )
                    ets, esum = head_front(hd, qt_h, kt_h)
                    if prev is not None:
                        head_tail(*prev)
                    prev = (hd, ets, esum, h2t)
                head_tail(*prev)

    # Full bacc pipeline: splits multi-waits into EventSemaphores, allocates
    # registers, and legalizes the module for walrus codegen.
    nc.compile()
    return nc


def _attention_bass(h, mask, Wq, bq, Wk, bk, Wv, bv):
    from concourse.bass_utils import run_bass_kernel_spmd

    if "nc" not in _NC_CACHE:
        _NC_CACHE["nc"] = _build_attention_nc()
    nc = _NC_CACHE["nc"]

    ht = np.ascontiguousarray(h.transpose(0, 2, 1))  # [B, 1280, 512] fp32
    hth = ht.astype(np.float16)
    wqt = np.ascontiguousarray(Wq.T).astype(np.float16)
    wkt = np.ascontiguousarray(Wk.T).astype(np.float16)
    wvt = np.ascontiguousarray(Wv.T).astype(np.float16)
    bqs = np.ascontiguousarray((bq * SCALE).reshape(NK, 128).T).astype(np.float32)
    bkt = np.ascontiguousarray(bk.reshape(NK, 128).T).astype(np.float32)
    bvt = np.ascontiguousarray(bv.reshape(NK, 128).T).astype(np.float32)
    maskt = np.ascontiguousarray(
        mask.reshape(B, NSC, 128).transpose(0, 2, 1)
    ).astype(np.float32)

    in_maps = []
    for c in range(N_CORES):
        sl = slice(c * BPC, (c + 1) * BPC)
        in_maps.append(
            dict(
                hth=hth[sl], htf=ht[sl], wqt=wqt, wkt=wkt, wvt=wvt,
                bqs=bqs, bkt=bkt, bvt=bvt,
                maskt=maskt[sl],
            )
        )
    res = run_bass_kernel_spmd(nc, in_maps, core_ids=list(range(N_CORES)))
    global _LAST_RES
    _LAST_RES = res
    outt = np.concatenate([r["outt"] for r in res.results], axis=0)  # [16,1280,512]
    return np.ascontiguousarray(outt.transpose(0, 2, 1))


def kernel(c_a_embeds, c_mask, c_lengths, Wih0, Whh0, b0, Wih1, Whh1, b1,
           Wq, bq, Wk, bk, Wv, bv):
    c_a_embeds = np.asarray(c_a_embeds, np.float32)
    lengths = np.asarray(c_lengths)
    mask2d = np.asarray(c_mask, np.float32).reshape(B, S)

    h = _bilstm_layer(c_a_embeds, np.asarray(Wih0), np.asarray(Whh0),
                      np.asarray(b0), lengths)
    h = _bilstm_layer(h, np.asarray(Wih1), np.asarray(Whh1),
                      np.asarray(b1), lengths)

    try:
        out = _attention_bass(h, mask2d, np.asarray(Wq), np.asarray(bq),
                              np.asarray(Wk), np.asarray(bk),
                              np.asarray(Wv), np.asarray(bv))
    except Exception as e:  # pragma: no cover - fallback path
        print(f"[kernel] bass attention failed ({type(e).__name__}: {e}); "
              "falling back to numpy", file=sys.stderr)
        out = _attention_numpy(h, np.asarray(c_mask, np.float32),
                               np.asarray(Wq), np.asarray(bq),
                               np.asarray(Wk), np.asarray(bk),
                               np.asarray(Wv), np.asarray(bv))
    return out.astype(np.float32)


# revision 13
# speedup vs baseline: 1.0653x; 1.0653x over previous
"""Trainium2 kernel for CustomContextEncoderForQG (v3).

Host: the two BiLSTM layers (sequential recurrence, small batch) run on CPU.
Device: attention block (QKV projections + 10-head softmax attention +
residual) as a Bass/Tile SPMD kernel on 8 NeuronCores, data-parallel over
batch (2 sequences per core).

Layout strategy: everything stays in transposed [feature, seq] layout
end-to-end, so no PE transposes are needed:
  - Q.T, K.T computed as [d, seq] (d on partitions); epilogue (bias+scale)
    fused on ScalarE via activation(Identity, bias, scale)
  - V computed in natural [seq, d] layout directly (no bias; folded below)
  - scores computed as S.T [k, q] = K.T-slices.T @ Q.T (contract over d)
  - softmax over k = partition dim: exp fused with the per-partition key
    mask as the activation bias (no max subtraction -- scores are O(1)
    here and the -10000 mask underflows to exactly 0); denominator D via
    all-ones matmul (rows of the PSUM tile all equal D); 1/D via the
    single-op reciprocal_approx_fast
  - ctx.T [d, q] = V-slices.T @ E.T (contract over k), then
    out.T = ctx.T * (1/D) + (h.T + bv)  (bv folded analytically: softmax
    rows sum to 1, so P @ (V + 1 bv^T) = P @ V + bv; h.T + bv precomputed
    once per sequence on ScalarE)
  - head loop is software-pipelined: scores+exp of head i issue before the
    denominator/context matmuls of head i-1, so the PE never waits on exp

All matmuls run in fp16 (same PE rate as bf16, 8x finer mantissa -- keeps
max rel err vs the fp32 reference under 1e-2); accumulation is fp32 PSUM.
"""

import sys
import numpy as np

sys.path.insert(0, "/opt/trn_rl_repo")


B, S, D_MODEL, H, NHEADS = 16, 512, 768, 640, 10
D_ATT = 2 * H  # 1280
HEAD_DIM = D_ATT // NHEADS  # 128
N_CORES = 8
BPC = B // N_CORES  # 2 sequences per core
NK = D_ATT // 128  # 10 chunks of the 1280 dim
NSC = S // 128  # 4 seq chunks of 128
SCALE = float(1.0 / np.sqrt(HEAD_DIM))


def _sigmoid(x):
    return 1.0 / (1.0 + np.exp(-x))


def _lstm_dir(xp, Whh, lengths, reverse):
    # xp: [B,S,4H]; packed-sequence semantics (state frozen, output zeroed
    # for t >= length); torch gate order i,f,g,o.
    Bs, Ss, H4 = xp.shape
    Hh = H4 // 4
    WhhT = np.ascontiguousarray(Whh.T)
    h = np.zeros((Bs, Hh), np.float32)
    c = np.zeros((Bs, Hh), np.float32)
    out = np.zeros((Bs, Ss, Hh), np.float32)
    ts = range(Ss - 1, -1, -1) if reverse else range(Ss)
    for t in ts:
        g = xp[:, t] + h @ WhhT
        i = _sigmoid(g[:, :Hh])
        f = _sigmoid(g[:, Hh : 2 * Hh])
        gg = np.tanh(g[:, 2 * Hh : 3 * Hh])
        o = _sigmoid(g[:, 3 * Hh :])
        c2 = f * c + i * gg
        h2 = o * np.tanh(c2)
        valid = (t < lengths)[:, None]
        h = np.where(valid, h2, h)
        c = np.where(valid, c2, c)
        out[:, t] = np.where(valid, h, 0.0)
    return out


def _bilstm_layer(x, Wih, Whh, b, lengths):
    outs = []
    for d, rev in ((0, False), (1, True)):
        xp = x @ Wih[d].T + b[d]
        outs.append(_lstm_dir(xp, Whh[d], lengths, rev))
    return np.concatenate(outs, axis=-1)


def _attention_numpy(h, mask, Wq, bq, Wk, bk, Wv, bv):
    q = (h @ Wq.T + bq).reshape(B, S, NHEADS, HEAD_DIM)
    k = (h @ Wk.T + bk).reshape(B, S, NHEADS, HEAD_DIM)
    v = (h @ Wv.T + bv).reshape(B, S, NHEADS, HEAD_DIM)
    scores = np.einsum("bqhd,bkhd->bhqk", q, k) / np.float32(np.sqrt(HEAD_DIM))
    scores = scores + mask  # [B,1,1,S]
    scores = scores - scores.max(-1, keepdims=True)
    e = np.exp(scores)
    probs = e / e.sum(-1, keepdims=True)
    ctx = np.einsum("bhqk,bkhd->bqhd", probs, v).reshape(B, S, D_ATT)
    return h + ctx


_NC_CACHE = {}
_LAST_RES = None


def _build_attention_nc(merge01):
    """merge01: exp over k-chunks 0-1 as one [128,1024] op with zero bias.
    Only valid when no sequence masks any key position < 256 (always true
    for lengths >= S/2); the caller checks the actual mask."""
    import concourse.bacc as bacc
    import concourse.mybir as mybir
    from concourse import tile

    fp32 = mybir.dt.float32
    f16 = mybir.dt.float16
    AF = mybir.ActivationFunctionType
    OP = mybir.AluOpType

    # Bacc (not plain Bass) so nc.compile() below can legalize the module
    # (walrus allows at most one semaphore wait per engine instruction).
    nc = bacc.Bacc()
    # fp16 h.T for matmul inputs; fp32 h.T for the residual path
    hth_ext = nc.declare_dram_parameter("hth", [BPC, D_ATT, S], f16, isOutput=False)
    htf_ext = nc.declare_dram_parameter("htf", [BPC, D_ATT, S], fp32, isOutput=False)
    wqt_ext = nc.declare_dram_parameter("wqt", [D_ATT, D_ATT], f16, isOutput=False)
    wkt_ext = nc.declare_dram_parameter("wkt", [D_ATT, D_ATT], f16, isOutput=False)
    wvt_ext = nc.declare_dram_parameter("wvt", [D_ATT, D_ATT], f16, isOutput=False)
    # bqs = bq * SCALE (folded into the Q epilogue's activation bias)
    bqs_ext = nc.declare_dram_parameter("bqs", [128, NK], fp32, isOutput=False)
    bkt_ext = nc.declare_dram_parameter("bkt", [128, NK], fp32, isOutput=False)
    bvt_ext = nc.declare_dram_parameter("bvt", [128, NK], fp32, isOutput=False)
    maskt_ext = nc.declare_dram_parameter("maskt", [BPC, 128, NSC], fp32, isOutput=False)
    out_ext = nc.declare_dram_parameter("outt", [BPC, D_ATT, S], fp32, isOutput=True)

    V_OCHUNKS = [(0, 512), (512, 512), (1024, 256)]

    with tile.TileContext(nc) as tc:
        with (
            tc.tile_pool(name="wpool", bufs=1) as wpool,
            tc.tile_pool(name="const", bufs=1) as const,
            tc.tile_pool(name="maskp", bufs=2) as maskp,
            tc.tile_pool(name="hb", bufs=1) as hb,
            tc.tile_pool(name="hf", bufs=1) as hf,
            tc.tile_pool(name="h2", bufs=1) as h2p,
            tc.tile_pool(name="qk", bufs=1) as qk,
            tc.tile_pool(name="vp", bufs=1) as vp,
            tc.tile_pool(name="et", bufs=4) as etp,
            tc.tile_pool(name="e2", bufs=4) as etp2,
            tc.tile_pool(name="rp", bufs=2) as rp,
            tc.tile_pool(name="op", bufs=3) as op,
            tc.tile_pool(name="ps", bufs=6, space="PSUM") as psp,
            tc.tile_pool(name="p2", bufs=1, space="PSUM") as psp2,
        ):
            ones = const.tile([128, 128], f16, tag="ones")
            nc.vector.memset(ones[:], 1.0)

            # --- DMA order: earliest phase (V: hth seq0 + wv) first ---
            hth = [[None] * NK for _ in range(BPC)]
            htf = [[None] * NK for _ in range(BPC)]
            wsb = {"q": [], "k": [], "v": []}
            for kc in range(NK):
                t = hb.tile([128, S], f16, tag=f"hth0_{kc}")
                nc.sync.dma_start(out=t[:], in_=hth_ext[0, kc * 128 : (kc + 1) * 128, :])
                hth[0][kc] = t
                wt = wpool.tile([128, D_ATT], f16, tag=f"wv{kc}")
                nc.sync.dma_start(out=wt[:], in_=wvt_ext[kc * 128 : (kc + 1) * 128, :])
                wsb["v"].append(wt)
            for name, wext in (("q", wqt_ext), ("k", wkt_ext)):
                for kc in range(NK):
                    wt = wpool.tile([128, D_ATT], f16, tag=f"w{name}{kc}")
                    nc.sync.dma_start(out=wt[:], in_=wext[kc * 128 : (kc + 1) * 128, :])
                    wsb[name].append(wt)
            bqs = const.tile([128, NK], fp32, tag="bqs")
            nc.sync.dma_start(out=bqs[:], in_=bqs_ext[:, :])
            bkt = const.tile([128, NK], fp32, tag="bkt")
            nc.sync.dma_start(out=bkt[:], in_=bkt_ext[:, :])
            bvt = const.tile([128, NK], fp32, tag="bvt")
            nc.sync.dma_start(out=bvt[:], in_=bvt_ext[:, :])
            masks = []
            for b in range(BPC):
                m = maskp.tile([128, NSC], fp32, tag="mask")
                nc.sync.dma_start(out=m[:], in_=maskt_ext[b])
                masks.append(m)
            for kc in range(NK):
                t = hf.tile([128, S], fp32, tag=f"htf0_{kc}")
                nc.sync.dma_start(out=t[:], in_=htf_ext[0, kc * 128 : (kc + 1) * 128, :])
                htf[0][kc] = t
            for kc in range(NK):
                t = hb.tile([128, S], f16, tag=f"hth1_{kc}")
                nc.sync.dma_start(out=t[:], in_=hth_ext[1, kc * 128 : (kc + 1) * 128, :])
                hth[1][kc] = t
            for kc in range(NK):
                t = hf.tile([128, S], fp32, tag=f"htf1_{kc}")
                nc.sync.dma_start(out=t[:], in_=htf_ext[1, kc * 128 : (kc + 1) * 128, :])
                htf[1][kc] = t

            for b in range(BPC):
                # --- V in natural [seq, d] layout (no bias; folded into hb2) ---
                v_tiles = []
                for sc in range(NSC):
                    vt = vp.tile([128, D_ATT], f16, tag=f"v{sc}")
                    for o0, on in V_OCHUNKS:
                        ps = psp.tile([128, S], fp32, tag="ps")
                        for kc in range(NK):
                            nc.tensor.matmul(
                                ps[:, :on],
                                hth[b][kc][:, sc * 128 : (sc + 1) * 128],
                                wsb["v"][kc][:, o0 : o0 + on],
                                start=(kc == 0),
                                stop=(kc == NK - 1),
                            )
                        nc.vector.tensor_copy(out=vt[:, o0 : o0 + on], in_=ps[:, :on])
                    v_tiles.append(vt)

                # --- Q.T, K.T projections in [d, seq] layout; epilogues on ACT ---
                qt_tiles = []
                kt_tiles = []
                for which, wlist, btile, scale, outlist in (
                    ("qT", wsb["q"], bqs, SCALE, qt_tiles),
                    ("kT", wsb["k"], bkt, 1.0, kt_tiles),
                ):
                    for mc in range(NK):
                        ps = psp.tile([128, S], fp32, tag="ps")
                        for kc in range(NK):
                            nc.tensor.matmul(
                                ps[:],
                                wlist[kc][:, mc * 128 : (mc + 1) * 128],
                                hth[b][kc][:],
                                start=(kc == 0),
                                stop=(kc == NK - 1),
                            )
                        ob = qk.tile([128, S], f16, tag=f"{which}{mc}")
                        nc.scalar.activation(
                            out=ob[:], in_=ps[:], func=AF.Identity,
                            bias=btile[:, mc : mc + 1], scale=scale,
                        )
                        outlist.append(ob)
                    # h.T + bv residual prep rides along on ACT
                    if which == "kT":
                        hb2 = []
                        for kc in range(NK):
                            h2t = h2p.tile([128, S], fp32, tag=f"hb2_{kc}")
                            nc.scalar.activation(
                                out=h2t[:], in_=htf[b][kc][:], func=AF.Identity,
                                bias=bvt[:, kc : kc + 1], scale=1.0,
                            )
                            hb2.append(h2t)

                # --- per-head attention, software-pipelined fronts/tails ---
                def head_front(hd):
                    """scores + exp for head hd -> E.T slices [k,q] per chunk"""
                    qt_h = qt_tiles[hd]
                    kt_h = kt_tiles[hd]
                    ets = []
                    if merge01:
                        # k-chunks 0,1: one 2-bank psum + one [128,1024] exp
                        # (zero bias -- those keys are never masked)
                        sps2 = psp2.tile([128, 2 * S], fp32, tag="ps2")
                        for kc4 in (0, 1):
                            nc.tensor.matmul(
                                sps2[:, kc4 * S : (kc4 + 1) * S],
                                kt_h[:, kc4 * 128 : (kc4 + 1) * 128],
                                qt_h[:],
                                start=True,
                                stop=True,
                            )
                        et2 = etp2.tile([128, 2 * S], f16, tag="et2")
                        nc.scalar.activation(
                            out=et2[:], in_=sps2[:], func=AF.Exp,
                            bias=0.0, scale=1.0,
                        )
                        ets.append(et2[:, 0:S])
                        ets.append(et2[:, S : 2 * S])
                        rest = (2, 3)
                    else:
                        rest = (0, 1, 2, 3)
                    for kc4 in rest:
                        sps = psp.tile([128, S], fp32, tag="ps")
                        nc.tensor.matmul(
                            sps[:],
                            kt_h[:, kc4 * 128 : (kc4 + 1) * 128],
                            qt_h[:],
                            start=True,
                            stop=True,
                        )
                        et = etp.tile([128, S], f16, tag="et")
                        nc.scalar.activation(
                            out=et[:], in_=sps[:], func=AF.Exp,
                            bias=masks[b][:, kc4 : kc4 + 1], scale=1.0,
                        )
                        ets.append(et[:])
                    return ets

                def head_tail(hd, ets):
                    """denominator, context, epilogue + store for head hd"""
                    dps = psp.tile([128, S], fp32, tag="ps")
                    for kc4 in range(NSC):
                        nc.tensor.matmul(
                            dps[:], ones[:], ets[kc4],
                            start=(kc4 == 0), stop=(kc4 == NSC - 1),
                        )
                    cps = psp.tile([128, S], fp32, tag="ps")
                    for kc4 in range(NSC):
                        nc.tensor.matmul(
                            cps[:],
                            v_tiles[kc4][:, hd * 128 : (hd + 1) * 128],
                            ets[kc4],
                            start=(kc4 == 0), stop=(kc4 == NSC - 1),
                        )
                    r = rp.tile([128, S], fp32, tag="r")
                    nc.vector.reciprocal_approx_fast(out=r[:], in_=dps[:])
                    ot = op.tile([128, S], fp32, tag="ot")
                    nc.vector.tensor_tensor(
                        out=ot[:], in0=cps[:], in1=r[:], op=OP.mult
                    )
                    nc.vector.tensor_tensor(
                        out=ot[:], in0=ot[:], in1=hb2[hd][:], op=OP.add
                    )
                    nc.sync.dma_start(
                        out=out_ext[b, hd * 128 : (hd + 1) * 128, :], in_=ot[:]
                    )

                prev = None
                for hd in range(NHEADS):
                    ets = head_front(hd)
                    if prev is not None:
                        head_tail(*prev)
                    prev = (hd, ets)
                head_tail(*prev)

    # Full bacc pipeline: splits multi-waits into EventSemaphores, allocates
    # registers, and legalizes the module for walrus codegen.
    nc.compile()
    return nc


def _attention_bass(h, mask, Wq, bq, Wk, bk, Wv, bv):
    from concourse.bass_utils import run_bass_kernel_spmd

    merge01 = bool(np.all(mask[:, : S // 2] == 0.0))
    key = ("nc", merge01)
    if key not in _NC_CACHE:
        _NC_CACHE[key] = _build_attention_nc(merge01)
    nc = _NC_CACHE[key]

    ht = np.ascontiguousarray(h.transpose(0, 2, 1))  # [B, 1280, 512] fp32
    hth = ht.astype(np.float16)
    wqt = np.ascontiguousarray(Wq.T).astype(np.float16)
    wkt = np.ascontiguousarray(Wk.T).astype(np.float16)
    wvt = np.ascontiguousarray(Wv.T).astype(np.float16)
    bqs = np.ascontiguousarray((bq * SCALE).reshape(NK, 128).T).astype(np.float32)
    bkt = np.ascontiguousarray(bk.reshape(NK, 128).T).astype(np.float32)
    bvt = np.ascontiguousarray(bv.reshape(NK, 128).T).astype(np.float32)
    maskt = np.ascontiguousarray(
        mask.reshape(B, NSC, 128).transpose(0, 2, 1)
    ).astype(np.float32)

    in_maps = []
    for c in range(N_CORES):
        sl = slice(c * BPC, (c + 1) * BPC)
        in_maps.append(
            dict(
                hth=hth[sl], htf=ht[sl], wqt=wqt, wkt=wkt, wvt=wvt,
                bqs=bqs, bkt=bkt, bvt=bvt,
                maskt=maskt[sl],
            )
        )
    res = run_bass_kernel_spmd(nc, in_maps, core_ids=list(range(N_CORES)))
    global _LAST_RES
    _LAST_RES = res
    outt = np.concatenate([r["outt"] for r in res.results], axis=0)  # [16,1280,512]
    return np.ascontiguousarray(outt.transpose(0, 2, 1))


def kernel(c_a_embeds, c_mask, c_lengths, Wih0, Whh0, b0, Wih1, Whh1, b1,
           Wq, bq, Wk, bk, Wv, bv):
    c_a_embeds = np.asarray(c_a_embeds, np.float32)
    lengths = np.asarray(c_lengths)
    mask2d = np.asarray(c_mask, np.float32).reshape(B, S)

    h = _bilstm_layer(c_a_embeds, np.asarray(Wih0), np.asarray(Whh0),
                      np.asarray(b0), lengths)
    h = _bilstm_layer(h, np.asarray(Wih1), np.asarray(Whh1),
                      np.asarray(b1), lengths)

    try:
        out = _attention_bass(h, mask2d, np.asarray(Wq), np.asarray(bq),
                              np.asarray(Wk), np.asarray(bk),
                              np.asarray(Wv), np.asarray(bv))
    except Exception as e:  # pragma: no cover - fallback path
        print(f"[kernel] bass attention failed ({type(e).__name__}: {e}); "
              "falling back to numpy", file=sys.stderr)
        out = _attention_numpy(h, np.asarray(c_mask, np.float32),
                               np.asarray(Wq), np.asarray(bq),
                               np.asarray(Wk), np.asarray(bk),
                               np.asarray(Wv), np.asarray(bv))
    return out.astype(np.float32)


# revision 14
# speedup vs baseline: 1.0685x; 1.0030x over previous
"""Trainium2 kernel for CustomContextEncoderForQG (v3).

Host: the two BiLSTM layers (sequential recurrence, small batch) run on CPU.
Device: attention block (QKV projections + 10-head softmax attention +
residual) as a Bass/Tile SPMD kernel on 8 NeuronCores, data-parallel over
batch (2 sequences per core).

Layout strategy: everything stays in transposed [feature, seq] layout
end-to-end, so no PE transposes are needed:
  - Q.T, K.T computed as [d, seq] (d on partitions); epilogue (bias+scale)
    fused on ScalarE via activation(Identity, bias, scale)
  - V computed in natural [seq, d] layout directly (no bias; folded below)
  - scores computed as S.T [k, q] = K.T-slices.T @ Q.T (contract over d)
  - softmax over k = partition dim: exp fused with the per-partition key
    mask as the activation bias (no max subtraction -- scores are O(1)
    here and the -10000 mask underflows to exactly 0); denominator D via
    all-ones matmul (rows of the PSUM tile all equal D); 1/D via the
    single-op reciprocal_approx_fast
  - ctx.T [d, q] = V-slices.T @ E.T (contract over k), then
    out.T = ctx.T * (1/D) + (h.T + bv)  (bv folded analytically: softmax
    rows sum to 1, so P @ (V + 1 bv^T) = P @ V + bv; h.T + bv precomputed
    once per sequence on ScalarE)
  - head loop is software-pipelined: scores+exp of head i issue before the
    denominator/context matmuls of head i-1, so the PE never waits on exp

All matmuls run in fp16 (same PE rate as bf16, 8x finer mantissa -- keeps
max rel err vs the fp32 reference under 1e-2); accumulation is fp32 PSUM.
"""

import sys
import numpy as np

sys.path.insert(0, "/opt/trn_rl_repo")


B, S, D_MODEL, H, NHEADS = 16, 512, 768, 640, 10
D_ATT = 2 * H  # 1280
HEAD_DIM = D_ATT // NHEADS  # 128
N_CORES = 8
BPC = B // N_CORES  # 2 sequences per core
NK = D_ATT // 128  # 10 chunks of the 1280 dim
NSC = S // 128  # 4 seq chunks of 128
SCALE = float(1.0 / np.sqrt(HEAD_DIM))


def _sigmoid(x):
    return 1.0 / (1.0 + np.exp(-x))


def _lstm_dir(xp, Whh, lengths, reverse):
    # xp: [B,S,4H]; packed-sequence semantics (state frozen, output zeroed
    # for t >= length); torch gate order i,f,g,o.
    Bs, Ss, H4 = xp.shape
    Hh = H4 // 4
    WhhT = np.ascontiguousarray(Whh.T)
    h = np.zeros((Bs, Hh), np.float32)
    c = np.zeros((Bs, Hh), np.float32)
    out = np.zeros((Bs, Ss, Hh), np.float32)
    ts = range(Ss - 1, -1, -1) if reverse else range(Ss)
    for t in ts:
        g = xp[:, t] + h @ WhhT
        i = _sigmoid(g[:, :Hh])
        f = _sigmoid(g[:, Hh : 2 * Hh])
        gg = np.tanh(g[:, 2 * Hh : 3 * Hh])
        o = _sigmoid(g[:, 3 * Hh :])
        c2 = f * c + i * gg
        h2 = o * np.tanh(c2)
        valid = (t < lengths)[:, None]
        h = np.where(valid, h2, h)
        c = np.where(valid, c2, c)
        out[:, t] = np.where(valid, h, 0.0)
    return out


def _bilstm_layer(x, Wih, Whh, b, lengths):
    outs = []
    for d, rev in ((0, False), (1, True)):
        xp = x @ Wih[d].T + b[d]
        outs.append(_lstm_dir(xp, Whh[d], lengths, rev))
    return np.concatenate(outs, axis=-1)


def _attention_numpy(h, mask, Wq, bq, Wk, bk, Wv, bv):
    q = (h @ Wq.T + bq).reshape(B, S, NHEADS, HEAD_DIM)
    k = (h @ Wk.T + bk).reshape(B, S, NHEADS, HEAD_DIM)
    v = (h @ Wv.T + bv).reshape(B, S, NHEADS, HEAD_DIM)
    scores = np.einsum("bqhd,bkhd->bhqk", q, k) / np.float32(np.sqrt(HEAD_DIM))
    scores = scores + mask  # [B,1,1,S]
    scores = scores - scores.max(-1, keepdims=True)
    e = np.exp(scores)
    probs = e / e.sum(-1, keepdims=True)
    ctx = np.einsum("bhqk,bkhd->bqhd", probs, v).reshape(B, S, D_ATT)
    return h + ctx


_NC_CACHE = {}
_LAST_RES = None


def _build_attention_nc(merge01):
    """merge01: exp over k-chunks 0-1 as one [128,1024] op with zero bias.
    Only valid when no sequence masks any key position < 256 (always true
    for lengths >= S/2); the caller checks the actual mask."""
    import concourse.bacc as bacc
    import concourse.mybir as mybir
    from concourse import tile

    fp32 = mybir.dt.float32
    f16 = mybir.dt.float16
    AF = mybir.ActivationFunctionType
    OP = mybir.AluOpType

    # Bacc (not plain Bass) so nc.compile() below can legalize the module
    # (walrus allows at most one semaphore wait per engine instruction).
    nc = bacc.Bacc()
    # fp16 h.T for matmul inputs; fp32 h.T for the residual path
    hth_ext = nc.declare_dram_parameter("hth", [BPC, D_ATT, S], f16, isOutput=False)
    htf_ext = nc.declare_dram_parameter("htf", [BPC, D_ATT, S], fp32, isOutput=False)
    wqt_ext = nc.declare_dram_parameter("wqt", [D_ATT, D_ATT], f16, isOutput=False)
    wkt_ext = nc.declare_dram_parameter("wkt", [D_ATT, D_ATT], f16, isOutput=False)
    wvt_ext = nc.declare_dram_parameter("wvt", [D_ATT, D_ATT], f16, isOutput=False)
    # bqs = bq * SCALE (folded into the Q epilogue's activation bias)
    bqs_ext = nc.declare_dram_parameter("bqs", [128, NK], fp32, isOutput=False)
    bkt_ext = nc.declare_dram_parameter("bkt", [128, NK], fp32, isOutput=False)
    bvt_ext = nc.declare_dram_parameter("bvt", [128, NK], fp32, isOutput=False)
    maskt_ext = nc.declare_dram_parameter("maskt", [BPC, 128, NSC], fp32, isOutput=False)
    out_ext = nc.declare_dram_parameter("outt", [BPC, D_ATT, S], fp32, isOutput=True)

    V_OCHUNKS = [(0, 512), (512, 512), (1024, 256)]

    with tile.TileContext(nc) as tc:
        with (
            tc.tile_pool(name="wpool", bufs=1) as wpool,
            tc.tile_pool(name="const", bufs=1) as const,
            tc.tile_pool(name="maskp", bufs=2) as maskp,
            tc.tile_pool(name="hb", bufs=1) as hb,
            tc.tile_pool(name="hf", bufs=1) as hf,
            tc.tile_pool(name="h2", bufs=1) as h2p,
            tc.tile_pool(name="qk", bufs=1) as qk,
            tc.tile_pool(name="vp", bufs=1) as vp,
            tc.tile_pool(name="et", bufs=4) as etp,
            tc.tile_pool(name="e2", bufs=4) as etp2,
            tc.tile_pool(name="rp", bufs=2) as rp,
            tc.tile_pool(name="op", bufs=3) as op,
            tc.tile_pool(name="ps", bufs=6, space="PSUM") as psp,
            tc.tile_pool(name="p2", bufs=1, space="PSUM") as psp2,
        ):
            ones = const.tile([128, 128], f16, tag="ones")
            nc.vector.memset(ones[:], 1.0)

            # --- DMA order: earliest phase (V: hth seq0 + wv) first ---
            hth = [[None] * NK for _ in range(BPC)]
            htf = [[None] * NK for _ in range(BPC)]
            wsb = {"q": [], "k": [], "v": []}
            for kc in range(NK):
                t = hb.tile([128, S], f16, tag=f"hth0_{kc}")
                nc.sync.dma_start(out=t[:], in_=hth_ext[0, kc * 128 : (kc + 1) * 128, :])
                hth[0][kc] = t
                wt = wpool.tile([128, D_ATT], f16, tag=f"wv{kc}")
                nc.sync.dma_start(out=wt[:], in_=wvt_ext[kc * 128 : (kc + 1) * 128, :])
                wsb["v"].append(wt)
            for name, wext in (("q", wqt_ext), ("k", wkt_ext)):
                for kc in range(NK):
                    wt = wpool.tile([128, D_ATT], f16, tag=f"w{name}{kc}")
                    nc.sync.dma_start(out=wt[:], in_=wext[kc * 128 : (kc + 1) * 128, :])
                    wsb[name].append(wt)
            bqs = const.tile([128, NK], fp32, tag="bqs")
            nc.sync.dma_start(out=bqs[:], in_=bqs_ext[:, :])
            bkt = const.tile([128, NK], fp32, tag="bkt")
            nc.sync.dma_start(out=bkt[:], in_=bkt_ext[:, :])
            bvt = const.tile([128, NK], fp32, tag="bvt")
            nc.sync.dma_start(out=bvt[:], in_=bvt_ext[:, :])
            masks = []
            for b in range(BPC):
                m = maskp.tile([128, NSC], fp32, tag="mask")
                nc.sync.dma_start(out=m[:], in_=maskt_ext[b])
                masks.append(m)
            for kc in range(NK):
                t = hf.tile([128, S], fp32, tag=f"htf0_{kc}")
                nc.sync.dma_start(out=t[:], in_=htf_ext[0, kc * 128 : (kc + 1) * 128, :])
                htf[0][kc] = t
            for kc in range(NK):
                t = hb.tile([128, S], f16, tag=f"hth1_{kc}")
                nc.sync.dma_start(out=t[:], in_=hth_ext[1, kc * 128 : (kc + 1) * 128, :])
                hth[1][kc] = t
            for kc in range(NK):
                t = hf.tile([128, S], fp32, tag=f"htf1_{kc}")
                nc.sync.dma_start(out=t[:], in_=htf_ext[1, kc * 128 : (kc + 1) * 128, :])
                htf[1][kc] = t

            for b in range(BPC):
                # --- V in natural [seq, d] layout (no bias; folded into hb2) ---
                v_tiles = []
                for sc in range(NSC):
                    vt = vp.tile([128, D_ATT], f16, tag=f"v{sc}")
                    for o0, on in V_OCHUNKS:
                        ps = psp.tile([128, S], fp32, tag="ps")
                        for kc in range(NK):
                            nc.tensor.matmul(
                                ps[:, :on],
                                hth[b][kc][:, sc * 128 : (sc + 1) * 128],
                                wsb["v"][kc][:, o0 : o0 + on],
                                start=(kc == 0),
                                stop=(kc == NK - 1),
                            )
                        nc.vector.tensor_copy(out=vt[:, o0 : o0 + on], in_=ps[:, :on])
                    v_tiles.append(vt)

                # --- Q.T, K.T projections in [d, seq] layout; epilogues on ACT ---
                qt_tiles = []
                kt_tiles = []
                for which, wlist, btile, scale, outlist in (
                    ("qT", wsb["q"], bqs, SCALE, qt_tiles),
                    ("kT", wsb["k"], bkt, 1.0, kt_tiles),
                ):
                    for mc in range(NK):
                        ps = psp.tile([128, S], fp32, tag="ps")
                        for kc in range(NK):
                            nc.tensor.matmul(
                                ps[:],
                                wlist[kc][:, mc * 128 : (mc + 1) * 128],
                                hth[b][kc][:],
                                start=(kc == 0),
                                stop=(kc == NK - 1),
                            )
                        ob = qk.tile([128, S], f16, tag=f"{which}{mc}")
                        nc.scalar.activation(
                            out=ob[:], in_=ps[:], func=AF.Identity,
                            bias=btile[:, mc : mc + 1], scale=scale,
                        )
                        outlist.append(ob)
                    # h.T + bv residual prep rides along on ACT
                    if which == "kT":
                        hb2 = []
                        for kc in range(NK):
                            h2t = h2p.tile([128, S], fp32, tag=f"hb2_{kc}")
                            nc.scalar.activation(
                                out=h2t[:], in_=htf[b][kc][:], func=AF.Identity,
                                bias=bvt[:, kc : kc + 1], scale=1.0,
                            )
                            hb2.append(h2t)

                # --- per-head attention, software-pipelined fronts/tails ---
                def head_front(hd):
                    """scores + exp for head hd -> E.T slices [k,q] per chunk"""
                    qt_h = qt_tiles[hd]
                    kt_h = kt_tiles[hd]
                    ets = []
                    if merge01:
                        # k-chunks 0,1: one 2-bank psum + one [128,1024] exp
                        # (zero bias -- those keys are never masked)
                        sps2 = psp2.tile([128, 2 * S], fp32, tag="ps2")
                        for kc4 in (0, 1):
                            nc.tensor.matmul(
                                sps2[:, kc4 * S : (kc4 + 1) * S],
                                kt_h[:, kc4 * 128 : (kc4 + 1) * 128],
                                qt_h[:],
                                start=True,
                                stop=True,
                            )
                        et2 = etp2.tile([128, 2 * S], f16, tag="et2")
                        nc.scalar.activation(
                            out=et2[:], in_=sps2[:], func=AF.Exp,
                            bias=0.0, scale=1.0,
                        )
                        ets.append(et2[:, 0:S])
                        ets.append(et2[:, S : 2 * S])
                        rest = (2, 3)
                    else:
                        rest = (0, 1, 2, 3)
                    for kc4 in rest:
                        sps = psp.tile([128, S], fp32, tag="ps")
                        nc.tensor.matmul(
                            sps[:],
                            kt_h[:, kc4 * 128 : (kc4 + 1) * 128],
                            qt_h[:],
                            start=True,
                            stop=True,
                        )
                        et = etp.tile([128, S], f16, tag="et")
                        nc.scalar.activation(
                            out=et[:], in_=sps[:], func=AF.Exp,
                            bias=masks[b][:, kc4 : kc4 + 1], scale=1.0,
                        )
                        ets.append(et[:])
                    return ets

                def head_tail(hd, ets):
                    """denominator, context, epilogue + store for head hd"""
                    dps = psp.tile([128, S], fp32, tag="ps")
                    for kc4 in range(NSC):
                        nc.tensor.matmul(
                            dps[:], ones[:], ets[kc4],
                            start=(kc4 == 0), stop=(kc4 == NSC - 1),
                        )
                    cps = psp.tile([128, S], fp32, tag="ps")
                    for kc4 in range(NSC):
                        nc.tensor.matmul(
                            cps[:],
                            v_tiles[kc4][:, hd * 128 : (hd + 1) * 128],
                            ets[kc4],
                            start=(kc4 == 0), stop=(kc4 == NSC - 1),
                        )
                    r = rp.tile([128, S], fp32, tag="r")
                    nc.vector.reciprocal_approx_fast(out=r[:], in_=dps[:])
                    ot = op.tile([128, S], fp32, tag="ot")
                    nc.vector.tensor_tensor(
                        out=ot[:], in0=cps[:], in1=r[:], op=OP.mult
                    )
                    nc.vector.tensor_tensor(
                        out=ot[:], in0=ot[:], in1=hb2[hd][:], op=OP.add
                    )
                    nc.sync.dma_start(
                        out=out_ext[b, hd * 128 : (hd + 1) * 128, :], in_=ot[:]
                    )

                prev = None
                for hd in range(NHEADS):
                    ets = head_front(hd)
                    if prev is not None:
                        head_tail(*prev)
                    prev = (hd, ets)
                head_tail(*prev)

    # Full bacc pipeline: splits multi-waits into EventSemaphores, allocates
    # registers, and legalizes the module for walrus codegen.
    nc.compile()
    return nc


def _attention_bass(h, mask, Wq, bq, Wk, bk, Wv, bv):
    from concourse.bass_utils import run_bass_kernel_spmd

    merge01 = bool(np.all(mask[:, : S // 2] == 0.0))
    key = ("nc", merge01)
    if key not in _NC_CACHE:
        _NC_CACHE[key] = _build_attention_nc(merge01)
    nc = _NC_CACHE[key]

    ht = np.ascontiguousarray(h.transpose(0, 2, 1))  # [B, 1280, 512] fp32
    hth = ht.astype(np.float16)
    wqt = np.ascontiguousarray(Wq.T).astype(np.float16)
    wkt = np.ascontiguousarray(Wk.T).astype(np.float16)
    wvt = np.ascontiguousarray(Wv.T).astype(np.float16)
    bqs = np.ascontiguousarray((bq * SCALE).reshape(NK, 128).T).astype(np.float32)
    bkt = np.ascontiguousarray(bk.reshape(NK, 128).T).astype(np.float32)
    bvt = np.ascontiguousarray(bv.reshape(NK, 128).T).astype(np.float32)
    maskt = np.ascontiguousarray(
        mask.reshape(B, NSC, 128).transpose(0, 2, 1)
    ).astype(np.float32)

    in_maps = []
    for c in range(N_CORES):
        sl = slice(c * BPC, (c + 1) * BPC)
        in_maps.append(
            dict(
                hth=hth[sl], htf=ht[sl], wqt=wqt, wkt=wkt, wvt=wvt,
                bqs=bqs, bkt=bkt, bvt=bvt,
                maskt=maskt[sl],
            )
        )
    res = run_bass_kernel_spmd(nc, in_maps, core_ids=list(range(N_CORES)))
    global _LAST_RES
    _LAST_RES = res
    outt = np.concatenate([r["outt"] for r in res.results], axis=0)  # [16,1280,512]
    return np.ascontiguousarray(outt.transpose(0, 2, 1))


def kernel(c_a_embeds, c_mask, c_lengths, Wih0, Whh0, b0, Wih1, Whh1, b1,
           Wq, bq, Wk, bk, Wv, bv):
    c_a_embeds = np.asarray(c_a_embeds, np.float32)
    lengths = np.asarray(c_lengths)
    mask2d = np.asarray(c_mask, np.float32).reshape(B, S)

    h = _bilstm_layer(c_a_embeds, np.asarray(Wih0), np.asarray(Whh0),
                      np.asarray(b0), lengths)
    h = _bilstm_layer(h, np.asarray(Wih1), np.asarray(Whh1),
                      np.asarray(b1), lengths)

    args = (h, mask2d, np.asarray(Wq), np.asarray(bq),
            np.asarray(Wk), np.asarray(bk), np.asarray(Wv), np.asarray(bv))
    try:
        out = _attention_bass(*args)
    except Exception as e:  # pragma: no cover - fallback paths
        print(f"[kernel] bass attention failed ({type(e).__name__}: {e}); "
              "retrying with tracing disabled", file=sys.stderr)
        try:
            import os

            os.environ["BASS_NEVER_TRACE"] = "1"
            out = _attention_bass(*args)
        except Exception as e2:
            print(f"[kernel] bass attention retry failed "
                  f"({type(e2).__name__}: {e2}); falling back to numpy",
                  file=sys.stderr)
            out = _attention_numpy(h, np.asarray(c_mask, np.float32),
                                   np.asarray(Wq), np.asarray(bq),
                                   np.asarray(Wk), np.asarray(bk),
                                   np.asarray(Wv), np.asarray(bv))
    return out.astype(np.float32)
